# revision 72
# baseline (speedup 1.0000x reference)
"""Trainium2 Bass kernel: single-head self-attention.

Reference computation (fp32):
    q = x @ Wq.T ; k = x @ Wk.T ; v = x @ Wv.T        (x: [4, 2048, 1024])
    out = softmax((q @ k.T) / 32) @ v                 ([4, 2048, 1024])

Sharding: 8 cores = (batch 4) x (sequence halves 2). Each core owns 1024
query rows of one batch element. No collectives: cross-core exchange is
avoided entirely by factoring BOTH sides of the attention through x:
    scores = (x Wq.T)(x Wk.T).T = x (Wq.T Wk) x.T = (x M) x.T
    out    = attn (x Wv.T)      = (attn x) Wv.T
so neither K nor V is ever materialized -- the stationary operands of the
big matmuls are x itself, which every core already holds for the full
sequence.

Every matmul runs fp8-e4m3 DoubleRow (0.5 cycles per moving row, 4x the
bf16 chain rate, 256-deep contraction planes). bf16-level accuracy where
needed comes from hi/lo operand splitting: X ~= X_hi + X_lo with
X_hi = fp8(X), X_lo = fp8(X - X_hi), so X (.) A = X_hi A_hi + X_hi A_lo
+ X_lo A_hi (the lo*lo term is ~2^-8 relative and dropped). A 3-pass
split matmul costs 0.75x the bf16 cycles at matching accuracy; all three
passes accumulate into one PSUM chain so drains are unchanged. The
attention-score side tolerates single-fp8 operands only where measured:
scores stay 1-pass (xt8 (.) yt8, error attenuated through softmax); M
and Y run 3-pass (2-pass variants measured over the 2e-2 gate). The V
side (Z = attn@x, C = z@Wv.T) is linear in operand error and runs
3-pass. Measured end-to-end rel-absmax 1.70e-2 vs the 2e-2 gate (the
bf16 baseline measured 1.80e-2).

All scale factors are powers of two (error-free): weights are quantized
at 32x (their +-1/32 range would be subnormal in e4m3), xr at 1/4 (so
zT = z/4 stays under the 240 e4m3 max), y drains at 2^-5, the exp
activation folds the combined 2^-10 into its scale argument, and the
denominator matmul uses ones=8.0 so the final per-query reciprocal
absorbs the z/4 * 32Wv = 8x output scale.

Per-core dataflow (all matmuls fp8 DoubleRow with fp32 PSUM):
  - hi/lo splits of all inputs are prepared host-side and shipped as
    merged multi-plane DRAM tensors so each SBUF tile loads with ONE
    wide DMA (28 input DMAs total -- HWDGE issue costs ~650 ns SEQ +
    625 ns descgen each, so many small plane loads would starve the PE;
    the weight quads split hi/lo so pass-0 starts ~1.75 us earlier).
  - M'[e,e'] = (32Wq).T (32Wk) 3-pass: batch 1 runs all 8 nch=0 tiles
    as concurrent PSUM groups (7 from the mm pool + 1 borrowed from the
    denominator bank) f-group-outermost at DMA arrival rate; batch 2
    (nch=1) chains after. Drained to m8 hi/lo by VectorE.
  - yT'[e',i] = M'8 (.) xt8 3-pass over own queries, ACT-drained at 2^-5
    straight to fp8 yt8 (single).
  - scoresT[j,i] = xt8hi (.) yt8, 1-pass; ScalarE applies exp(s * 2^-10)
    out of PSUM to a bf16 stage; VectorE extracts e8hi = fp8(E) and
    e8lo = fp8(E - e8hi). 5 zT accumulation groups are held in PSUM
    (lazily allocated) and their 3-pass matmuls plus the denominator
    matmuls are spread a few-per-chain behind the scores stream so the
    PE fills every exp-drain window. PSUM allows only one open
    accumulation chain per bank, so the denominator partials are
    single-shot chains into 64 distinct columns of the dn bank, reduced
    to [P, IT] by a VectorE tree-add.
  - zT[e,i] = xr8 (.) e8 3-pass over the full key sequence.
  - out[i,f] = (z8 (.) wv8 3-pass) * recip[i]; normalization folds into
    the ACT drain as a per-partition scale. Chunks whose z-columns drain
    last run mid-stream, and the stream closes on early-drained it3 with
    a 64-col chunk so the final drain+store tail is short.

Performance: 129.8 us TimelineSim (bf16 baseline: 158.4 us). ~120 us of
TensorE busy at 2.4 GHz; DMA (13 MB, fp8 everywhere) overlaps the M
phase; softmax/drains run on ScalarE/VectorE under the matmul stream
(hi-extractions on ACT, lo-subtracts on DVE -- a PSUM-reading DVE op
costs ~658 ns per [P,512] chunk and one engine alone would pace the
PE; ic1 e8-extractions defer one jt so the ic0 chunks that gate the
held-z fillers extract first); PE p-state warmup burns the clock ramp
inside the startup DMA window.
"""

import numpy as np
import ml_dtypes
from contextlib import ExitStack

import concourse.bacc as bacc
import concourse.tile as tile
import concourse.mybir as mybir

BF16 = mybir.dt.bfloat16
FP8 = mybir.dt.float8e4
F32 = mybir.dt.float32
P = 128
B, S, D = 4, 2048, 1024
SQ = S // 2   # query rows per core
N_CORES = 8
ET = D // P   # contraction tiles over embed dim
GE = ET // 2  # pair-plane groups over embed dim
JT = S // P   # kv-sequence tiles
GJ = JT // 2  # pair-plane groups over kv sequence
IT = SQ // P  # query tiles
NCH = 512     # moving-operand chunk (one fp32 PSUM bank)
DR = mybir.MatmulPerfMode.DoubleRow
NZH = 5       # zT accumulation groups held in PSUM under the scores loop

_CACHE: dict = {}


def _build(repeats=1, upto=99):
    nc = bacc.Bacc("TRN2", target_bir_lowering=False, debug=False, num_devices=N_CORES)
    # merged multi-plane inputs: one wide DMA per SBUF tile
    wqk8 = nc.dram_tensor("wqk8", [GE, P, 8, D], FP8, kind="ExternalInput").ap()
    xtq8 = nc.dram_tensor("xtq8", [GE, P, 4, SQ], FP8, kind="ExternalInput").ap()
    xtk8 = nc.dram_tensor("xtk8", [GE, P, 2, SQ], FP8, kind="ExternalInput").ap()
    xr8 = nc.dram_tensor("xr8", [GJ, P, 4, D], FP8, kind="ExternalInput").ap()
    wv8 = nc.dram_tensor("wv8", [GE, P, 4, D], FP8, kind="ExternalInput").ap()
    out = nc.dram_tensor("out", [SQ, D], F32, kind="ExternalOutput").ap()

    with tile.TileContext(nc) as tc, ExitStack() as ctx:
        w_pool = ctx.enter_context(tc.tile_pool(name="w", bufs=1))
        xt_pool = ctx.enter_context(tc.tile_pool(name="xt", bufs=1))
        xr_pool = ctx.enter_context(tc.tile_pool(name="xr", bufs=1))
        wv_pool = ctx.enter_context(tc.tile_pool(name="wv", bufs=1))
        m_pool = ctx.enter_context(tc.tile_pool(name="m", bufs=1))
        yt_pool = ctx.enter_context(tc.tile_pool(name="yt", bufs=1))
        e_pool = ctx.enter_context(tc.tile_pool(name="e", bufs=1))
        z_pool = ctx.enter_context(tc.tile_pool(name="z", bufs=1))
        estage_pool = ctx.enter_context(tc.tile_pool(name="estage", bufs=6))
        stage_pool = ctx.enter_context(tc.tile_pool(name="stage", bufs=6))
        small_pool = ctx.enter_context(tc.tile_pool(name="small", bufs=1))
        mm_psum = ctx.enter_context(tc.tile_pool(name="mmps", bufs=7, space="PSUM"))
        dn_psum = ctx.enter_context(tc.tile_pool(name="dnps", bufs=1, space="PSUM"))

        # ---- DMA loads: one dma_start per tile, issued on the SP queue in
        # first-use order (the serial DMA bus then transfers in this order).
        # hi planes land first so each group's pass-0 matmuls start ~1.75 us
        # earlier than a single 1 MB quad transfer would allow
        wqk_sb = []
        for g in range(GE):
            t = w_pool.tile([P, 8, D], FP8, name=f"wqk{g}")
            nc.sync.dma_start(t[:, 0:4, :], wqk8[g][:, 0:4, :])
            nc.sync.dma_start(t[:, 4:8, :], wqk8[g][:, 4:8, :])
            wqk_sb.append(t)
        xtq_sb, xtk_sb, xr_sb, wv_sb = [], [], [], []
        if upto >= 2:
            for g in range(GE):
                t = xt_pool.tile([P, 4, SQ], FP8, name=f"xtq{g}")
                nc.sync.dma_start(t[:], xtq8[g])
                xtq_sb.append(t)
        if upto >= 3:
            for g in range(GE):
                t = xt_pool.tile([P, 2, SQ], FP8, name=f"xtk{g}")
                nc.sync.dma_start(t[:], xtk8[g])
                xtk_sb.append(t)
            for gj in range(GJ):
                t = xr_pool.tile([P, 4, D], FP8, name=f"xr{gj}")
                nc.sync.dma_start(t[:], xr8[gj])
                xr_sb.append(t)
        if upto >= 5:
            for g in range(GE):
                t = wv_pool.tile([P, 4, D], FP8, name=f"wv{g}")
                nc.sync.dma_start(t[:], wv8[g])
                wv_sb.append(t)

        tensors = dict(wqk=wqk_sb, xtq=xtq_sb, xtk=xtk_sb, xr=xr_sb, wv=wv_sb)
        for _rep in range(repeats):
            _compute(nc, tensors, m_pool, yt_pool, e_pool, z_pool,
                     estage_pool, stage_pool, small_pool, mm_psum, dn_psum, out,
                     upto=upto)

    nc.compile()
    return nc


def _compute(nc, t, m_pool, yt_pool, e_pool, z_pool, estage_pool, stage_pool,
             small_pool, mm_psum, dn_psum, out, upto=99):
    Exp = mybir.ActivationFunctionType.Exp
    Copy = mybir.ActivationFunctionType.Copy
    sub = mybir.AluOpType.subtract
    # plane slices of the merged weight tiles
    QHI, KHI, QLO, KLO = slice(0, 2), slice(2, 4), slice(4, 6), slice(6, 8)
    HI, LO = slice(0, 2), slice(2, 4)

    # ---- Phase W: PE p-state warmup. The cost model (and HW) run the PE at
    # reduced clock until ~3 us of continuous busy; burn that ramp on dummy
    # const matmuls during the otherwise-idle startup DMA window.
    warm_c = nc.const_aps.tensor(1.0, (P, NCH), BF16)
    for w in range(8):
        psw = mm_psum.tile([P, NCH], F32, name="ps_w", tag="mm")
        nc.tensor.matmul(psw[:], warm_c[:, 0:P], warm_c[:], start=True, stop=True)
    ones8 = small_pool.tile([P, 2, 1], FP8, name="ones8")
    nc.vector.memset(ones8[:], 8.0)

    # ---- Phase M: M'[e, e'] = sum_f (32Wq)[f, e] (32Wk)[f, e'], 3-pass.
    # Batch 1 (nch=0) runs all 8 e-tiles as concurrent PSUM groups
    # f-group-outermost so the PE streams at the DMA arrival cadence; the
    # 8th group borrows the denominator bank (idle until phase B2).
    mhi_sb = [m_pool.tile([P, 2, D], FP8, name=f"mhi{g}") for g in range(GE)]
    mlo_sb = [m_pool.tile([P, 2, D], FP8, name=f"mlo{g}") for g in range(GE)]
    m_passes = [(QHI, KHI), (QHI, KLO), (QLO, KHI)]

    # hi extraction rides the otherwise-idle Activation engine; only the lo
    # subtract stays on VectorE (a PSUM-reading DVE op costs ~658 ns per
    # [P,512] chunk -- both on one engine would pace the whole M phase)
    def m_drain(mt, nch, ps):
        hi = mhi_sb[mt // 2][:, mt % 2, nch * NCH:(nch + 1) * NCH]
        lo = mlo_sb[mt // 2][:, mt % 2, nch * NCH:(nch + 1) * NCH]
        nc.scalar.activation(hi, ps[:], Copy, scale=1.0)
        nc.vector.tensor_tensor(lo, ps[:], hi, sub)

    # the denominator bank moonlights as the 8th M accumulation group (it
    # is idle until phase B2, and the start flag resets accumulation)
    ps_dn_bank = dn_psum.tile([P, NCH], F32, name="ps_dn")
    ps_a = [mm_psum.tile([P, NCH], F32, name=f"ps_mA{mt}", tag="mm")
            for mt in range(7)]
    ps_a.append(ps_dn_bank)
    def m_mm(ps, g, sq, sk, mt, nch, start, stop):
        nc.tensor.matmul(
            ps[:],
            t["wqk"][g][:, sq, mt * P:(mt + 1) * P],
            t["wqk"][g][:, sk, nch * NCH:(nch + 1) * NCH],
            start=start, stop=stop, perf_mode=DR,
        )

    for g in range(GE - 1):
        for pi, (sq, sk) in enumerate(m_passes):
            for mt in range(ET):
                m_mm(ps_a[mt], g, sq, sk, mt, 0, g == 0 and pi == 0, False)
    # last f-group runs tile-major with immediate drains so PSUM banks free
    # one by one (batch 2 starts ~2 us earlier than an all-tiles-at-g3 end)
    for mt in range(ET):
        for pi, (sq, sk) in enumerate(m_passes):
            m_mm(ps_a[mt], GE - 1, sq, sk, mt, 0, False, pi == len(m_passes) - 1)
        m_drain(mt, 0, ps_a[mt])
    for mt in range(ET):
        ps = mm_psum.tile([P, NCH], F32, name="ps_m", tag="mm")
        for pi, (sq, sk) in enumerate(m_passes):
            for g in range(GE):
                m_mm(ps, g, sq, sk, mt, 1,
                     g == 0 and pi == 0,
                     g == GE - 1 and pi == len(m_passes) - 1)
        m_drain(mt, 1, ps)

    if upto < 2:
        return
    # ---- Phase Y: yT'[e', i] = sum_e M'[e, e'] xt[e, i], 3-pass, queries
    # only; ACT-drained at 2^-5 straight to fp8 yt8 for the scores matmuls.
    yt8_sb = [yt_pool.tile([P, 2, SQ], FP8, name=f"yt8_{g}") for g in range(GE)]
    y_passes = [(mhi_sb, HI), (mlo_sb, HI), (mhi_sb, LO)]
    for ft in range(ET):
        for ic in range(SQ // NCH):
            ps = mm_psum.tile([P, NCH], F32, name="ps_y", tag="mm")
            for pi, (msb, xsl) in enumerate(y_passes):
                for g in range(GE):
                    nc.tensor.matmul(
                        ps[:],
                        msb[g][:, :, ft * P:(ft + 1) * P],
                        t["xtq"][g][:, xsl, ic * NCH:(ic + 1) * NCH],
                        start=(g == 0 and pi == 0),
                        stop=(g == GE - 1 and pi == len(y_passes) - 1),
                        perf_mode=DR,
                    )
            nc.scalar.activation(
                yt8_sb[ft // 2][:, ft % 2, ic * NCH:(ic + 1) * NCH],
                ps[:], Copy, scale=2.0 ** -5)

    if upto < 3:
        return
    # ---- Phase S: scoresT[j, i] = sum_e' xt8[e', j] yt8[e', i] (1-pass);
    # exp via ScalarE to a bf16 stage, VectorE extracts e8 hi/lo. NZH zT
    # accumulation groups are held in PSUM; their 3-pass matmuls are
    # emitted a few per scores chain (eligibility lags the exp drains by
    # one chain) so the PE fills every drain window.
    ehi_sb = [e_pool.tile([P, 2, SQ], FP8, name=f"ehi{gj}") for gj in range(GJ)]
    elo_sb = [e_pool.tile([P, 2, SQ], FP8, name=f"elo{gj}") for gj in range(GJ)]
    zhi_sb = [z_pool.tile([P, 2, SQ], FP8, name=f"zhi{g}") for g in range(GE)]
    zlo_sb = [z_pool.tile([P, 2, SQ], FP8, name=f"zlo{g}") for g in range(GE)]
    # allocated lazily at the first held-z matmul so the early filler-less
    # scores chains can rotate through all 7 mm banks
    ps_z = []
    z_passes = [(HI, ehi_sb), (LO, ehi_sb), (HI, elo_sb)]

    def z_mm(i, ic, gj, pi):
        if not ps_z:
            ps_z.extend(mm_psum.tile([P, NCH], F32, name=f"ps_zh{k}", tag="mm")
                        for k in range(NZH))
        xsl, esb = z_passes[pi]
        nc.tensor.matmul(
            ps_z[i][:],
            t["xr"][gj][:, xsl, i * P:(i + 1) * P],
            esb[gj][:, :, ic * NCH:(ic + 1) * NCH],
            start=(gj == 0 and pi == 0),
            stop=(gj == GJ - 1 and pi == len(z_passes) - 1),
            perf_mode=DR,
        )

    # Pending PE filler matmuls, emitted a few per scores chain: the held-z
    # accumulations plus the denominator chains (denomT[i(part), it] via
    # e8hi (.) 8.0, DoubleRow N=1 into the dn bank, one accumulation group
    # per column -- SEQ-bound, so they ride the stream's decode slack).
    # PSUM supports only one open accumulation chain per bank, so each
    # (it, gj) partial is its own single-shot chain into its own column
    # (col = it*GJ + gj); a VectorE tree-add reduces [P, IT*GJ] -> [P, IT].
    dn = ps_dn_bank

    def dn_mm(it, gj):
        col = it * GJ + gj
        nc.tensor.matmul(
            dn[:, col:col + 1],
            ehi_sb[gj][:, :, it * P:(it + 1) * P],
            ones8[:],
            start=True, stop=True,
            perf_mode=DR,
        )

    held = []
    for gj in range(GJ):
        if upto >= 4:
            held.extend(("z", gj, pi, i) for pi in range(3) for i in range(NZH))
        if upto >= 5:
            held.extend(("dn", gj, it, None) for it in range(IT))
    emitted = 0

    def emit_held(jt, ic, budget):
        nonlocal emitted
        # group gj's exp tiles are complete after chain (2gj+1, ic=1); the
        # one-full-chain lag keeps the PE from blocking on the exp drains
        while emitted < len(held) and budget > 0:
            kind, gj, a, b = held[emitted]
            if 2 * gj + 1 >= jt:
                return
            if kind == "z":
                z_mm(b, 0, gj, a)
            else:
                dn_mm(a, gj)
            emitted += 1
            budget -= 1

    # ic1 e8-extractions are deferred one jt on VectorE: the held-z fillers
    # gate only on the ic0 chunks, so pulling those forward keeps the PE's
    # filler queue from waiting on DVE early in the phase
    dve_deferred = []
    for jt in range(JT):
        for ic in range(SQ // NCH):
            ps = mm_psum.tile([P, NCH], F32, name="ps_s", tag="mm")
            for g in range(GE):
                stat = (t["xtq"][g][:, HI, jt * P:(jt + 1) * P] if jt < IT
                        else t["xtk"][g][:, :, (jt - IT) * P:(jt - IT + 1) * P])
                nc.tensor.matmul(
                    ps[:], stat,
                    yt8_sb[g][:, :, ic * NCH:(ic + 1) * NCH],
                    start=(g == 0), stop=(g == GE - 1),
                    perf_mode=DR,
                )
            est = estage_pool.tile([P, NCH], BF16, name="estage", tag="est")
            nc.scalar.activation(est[:], ps[:], Exp, scale=2.0 ** -10)
            ehi = ehi_sb[jt // 2][:, jt % 2, ic * NCH:(ic + 1) * NCH]
            elo = elo_sb[jt // 2][:, jt % 2, ic * NCH:(ic + 1) * NCH]
            if ic == 0:
                nc.vector.tensor_copy(ehi, est[:])
                nc.vector.tensor_tensor(elo, est[:], ehi, sub)
            else:
                dve_deferred.append((est, ehi, elo))
            emit_held(jt, ic, 4)
        while len(dve_deferred) > 1:
            dest, dehi, delo = dve_deferred.pop(0)
            nc.vector.tensor_copy(dehi, dest[:])
            nc.vector.tensor_tensor(delo, dest[:], dehi, sub)
    for dest, dehi, delo in dve_deferred:
        nc.vector.tensor_copy(dehi, dest[:])
        nc.vector.tensor_tensor(delo, dest[:], dehi, sub)
    while emitted < len(held):
        kind, gj, a, b = held[emitted]
        if kind == "z":
            z_mm(b, 0, gj, a)
        else:
            dn_mm(a, gj)
        emitted += 1
    if upto < 5:
        return
    add = mybir.AluOpType.add
    dnv = dn[:, 0:IT * GJ].rearrange("p (a b) -> p a b", a=IT, b=GJ)
    ds8 = small_pool.tile([P, IT, GJ], F32, name="ds8")
    ds4 = small_pool.tile([P, IT, 4], F32, name="ds4")
    ds2 = small_pool.tile([P, IT, 2], F32, name="ds2")
    denomT = small_pool.tile([P, IT], F32, name="denomT")
    recipT = small_pool.tile([P, IT], F32, name="recipT")
    nc.vector.tensor_copy(ds8[:], dnv)
    nc.vector.tensor_tensor(ds4[:], ds8[:, :, 0:4], ds8[:, :, 4:8], add)
    nc.vector.tensor_tensor(ds2[:], ds4[:, :, 0:2], ds4[:, :, 2:4], add)
    nc.vector.tensor_tensor(denomT[:], ds2[:, :, 0], ds2[:, :, 1], add)
    nc.vector.reciprocal(recipT[:], denomT[:])

    def z_drain(et, ic, ps):
        hi = zhi_sb[et // 2][:, et % 2, ic * NCH:(ic + 1) * NCH]
        lo = zlo_sb[et // 2][:, et % 2, ic * NCH:(ic + 1) * NCH]
        nc.scalar.activation(hi, ps[:], Copy, scale=1.0)
        nc.vector.tensor_tensor(lo, ps[:], hi, sub)

    for i in range(NZH):
        z_drain(i, 0, ps_z[i])

    # ---- Phase Z: remaining zT groups (the first NZH of ic=0 were computed
    # interleaved with the scores loop above)
    for et in range(ET):
        for ic in range(SQ // NCH):
            if ic == 0 and et < NZH:
                continue
            ps = mm_psum.tile([P, NCH], F32, name="ps_z", tag="mm")
            for pi in range(3):
                for gj in range(GJ):
                    xsl, esb = z_passes[pi]
                    nc.tensor.matmul(
                        ps[:],
                        t["xr"][gj][:, xsl, et * P:(et + 1) * P],
                        esb[gj][:, :, ic * NCH:(ic + 1) * NCH],
                        start=(gj == 0 and pi == 0),
                        stop=(gj == GJ - 1 and pi == 2),
                        perf_mode=DR,
                    )
            z_drain(et, ic, ps)

    # ---- Phase C: out[i, f] = (sum_e zT[e, i] wv[e, f]) * recip[i], 3-pass.
    # chunk order is free (all recips ready): chunks depending on the LAST
    # z-drain (it 4..7, ic1 region) run mid-stream so its ACT+sem latency
    # hides under earlier chains; the stream opens with early-drained it 0..2
    # and closes on it3 (ic0, drained long before) with a tiny 64-col chunk
    # so the final drain+store tail is short
    c_passes = [(zhi_sb, HI), (zhi_sb, LO), (zlo_sb, HI)]
    early = [0, 1, 2, 4, 5, 6, 7]
    chunks = [(it, fc * NCH, NCH) for it in early for fc in range(D // NCH)]
    chunks += [(3, 0, NCH), (3, NCH, 448), (3, D - 64, 64)]
    for ci, (it, f0, fw) in enumerate(chunks):
        ps = mm_psum.tile([P, fw], F32, name="ps_o", tag="mm")
        for pi, (zsb, wsl) in enumerate(c_passes):
            for g in range(GE):
                nc.tensor.matmul(
                    ps[:],
                    zsb[g][:, :, it * P:(it + 1) * P],
                    t["wv"][g][:, wsl, f0:f0 + fw],
                    start=(g == 0 and pi == 0),
                    stop=(g == GE - 1 and pi == len(c_passes) - 1),
                    perf_mode=DR,
                )
        st = stage_pool.tile([P, fw], F32, name="ostage", tag="ostage")
        nc.scalar.activation(st[:], ps[:], Copy, scale=recipT[:, it:it + 1])
        nc.sync.dma_start(out[it * P:(it + 1) * P, f0:f0 + fw], st[:])


def _get_nc(repeats=1):
    key = ("nc", repeats)
    if key not in _CACHE:
        _CACHE[key] = _build(repeats)
    return _CACHE[key]


def _prep_inputs(x, Wq, Wk, Wv):
    fp8 = ml_dtypes.float8_e4m3
    f32 = np.float32

    def split8(a):
        hi = a.astype(fp8)
        lo = (a - hi.astype(f32)).astype(fp8)
        return hi, lo

    def pair_planes(*mats):
        """Stack [D_rows, C] fp8 mats into [groups, P, nplanes, C]: for each
        row-pair-group g, planes are (m0 rows 2g, m0 rows 2g+1, m1 rows 2g,
        m1 rows 2g+1, ...)."""
        rows = mats[0].shape[0]
        ng = rows // (2 * P)
        cols = mats[0].shape[1]
        outp = np.empty((ng, P, 2 * len(mats), cols), fp8)
        for g in range(ng):
            for mi, m in enumerate(mats):
                outp[:, :, 2 * mi + 0, :][g] = m[(2 * g) * P:(2 * g + 1) * P]
                outp[:, :, 2 * mi + 1, :][g] = m[(2 * g + 1) * P:(2 * g + 2) * P]
        return outp

    x = np.asarray(x, dtype=f32)
    wq_hi, wq_lo = split8(32.0 * np.asarray(Wq, f32))
    wk_hi, wk_lo = split8(32.0 * np.asarray(Wk, f32))
    wv_hi, wv_lo = split8(np.ascontiguousarray(32.0 * np.asarray(Wv, f32).T))
    wqk = pair_planes(wq_hi, wk_hi, wq_lo, wk_lo)   # [GE, P, 8, D]
    wv8 = pair_planes(wv_hi, wv_lo)                 # [GE, P, 4, D]
    in_maps = []
    for c in range(N_CORES):
        b, h = divmod(c, 2)
        xb = x[b]  # [S, D]
        # this core's query half first, then the other half (j-order is a
        # consistent permutation of the keys and values, so attention is
        # unaffected)
        xr = np.concatenate([xb[h * SQ:(h + 1) * SQ], xb[(1 - h) * SQ:(2 - h) * SQ]],
                            axis=0)          # [S, D]
        xt = np.ascontiguousarray(xr.T)      # [D, S]
        xt_hi, xt_lo = split8(xt)
        xr_hi, xr_lo = split8(np.ascontiguousarray(xr) * 0.25)
        in_maps.append({
            "wqk8": wqk,
            "xtq8": pair_planes(xt_hi[:, 0:SQ], xt_lo[:, 0:SQ]),  # [GE, P, 4, SQ]
            "xtk8": pair_planes(xt_hi[:, SQ:S]),                  # [GE, P, 2, SQ]
            "xr8": pair_planes(xr_hi, xr_lo),                     # [GJ, P, 4, D]
            "wv8": wv8,
        })
    return in_maps


def _get_runner():
    """Cached jitted dispatcher: one XLA/NEFF compile per process, reused
    across kernel() calls (run_bass_kernel_spmd would recompile per call)."""
    if "runner" in _CACHE:
        return _CACHE["runner"]
    import jax
    from jax.sharding import Mesh, PartitionSpec
    from jax.experimental.shard_map import shard_map
    from concourse.bass2jax import (
        _bass_exec_p, install_neuronx_cc_hook, partition_id_tensor)

    nc = _get_nc()
    install_neuronx_cc_hook()

    in_names, out_names, out_avals = [], [], []
    partition_name = nc.partition_id_tensor.name if nc.partition_id_tensor else None
    for alloc in nc.m.functions[0].allocations:
        if not isinstance(alloc, mybir.MemoryLocationSet):
            continue
        name = alloc.memorylocations[0].name
        if alloc.kind == "ExternalInput":
            if name != partition_name:
                in_names.append(name)
        elif alloc.kind == "ExternalOutput":
            out_names.append(name)
            out_avals.append(jax.core.ShapedArray(
                tuple(alloc.tensor_shape), mybir.dt.np(alloc.dtype)))
    n_params = len(in_names)
    all_names = list(in_names) + out_names
    if partition_name is not None:
        all_names.append(partition_name)

    def _body(*args):
        operands = list(args)
        if partition_name is not None:
            operands.append(partition_id_tensor())
        return tuple(_bass_exec_p.bind(
            *operands,
            out_avals=tuple(out_avals),
            in_names=tuple(all_names),
            out_names=tuple(out_names),
            lowering_input_output_aliases=(),
            sim_require_finite=True,
            sim_require_nnan=True,
            nc=nc,
        ))

    devices = jax.devices()[:N_CORES]
    mesh = Mesh(np.asarray(devices), ("core",))
    nspecs = (PartitionSpec("core"),) * (n_params + len(out_names))
    sharded = jax.jit(
        shard_map(_body, mesh=mesh, in_specs=nspecs,
                  out_specs=(PartitionSpec("core"),) * len(out_names),
                  check_rep=False),
        keep_unused=True,
    )

    def run(in_maps):
        concat_in = [
            np.concatenate([in_maps[c][name] for c in range(N_CORES)], axis=0)
            for name in in_names
        ]
        concat_zero = [
            np.zeros((N_CORES * a.shape[0], *a.shape[1:]), a.dtype)
            for a in out_avals
        ]
        outs = sharded(*concat_in, *concat_zero)
        return {
            name: np.asarray(outs[i]).reshape(N_CORES, *out_avals[i].shape)
            for i, name in enumerate(out_names)
        }

    _CACHE["runner"] = run
    return run


def kernel(x, Wq, Wk, Wv):
    in_maps = _prep_inputs(x, Wq, Wk, Wv)
    res = _get_runner()(in_maps)
    out = np.empty((B, S, D), dtype=np.float32)
    for c in range(N_CORES):
        b, h = divmod(c, 2)
        out[b, h * SQ:(h + 1) * SQ, :] = res["out"][c]
    return out
